# revision 1
# baseline (speedup 1.0000x reference)
"""Segment-softmax attention-scatter kernel for 8 Trainium2 NeuronCores.

Math (per reference):
    v = tanh(x @ W + b);  vu = v @ u;  e = exp(vu)        [no max-subtract:
        |vu| <= ||u||_1 ~ 28, exp never overflows fp32]
    alphas = e / segment_sum(e);  out = segment_sum(x * alphas)

Sharding: segments are split into 8 contiguous ranges (ids are sorted, so each
core's rows are one contiguous slice -> no cross-core reduction at all). Each
core owns G=ceil(S/8/128) groups of 128 segments. Host pads every (core,group)
row range to CH chunks of 128 rows so all 8 cores run one identical NEFF.

Per 512-row block (transposed phase A):
    preT[d',r] = sum_d W[d,d'] x[r,d]   (16 fp32r matmuls, W stationary)
    vT = tanh(preT + b)                  (ACT, bias is per-partition here)
    vu[1,r] += u_dk^T @ vT_dk            (4 fp32r matmuls)
    e_row = exp(vu)                      (ACT)
Per 128-row chunk:
    e_col = e_row-slice^T via K=1 matmul; mask = (iota == id-128g) on DVE;
    emask = mask * e_col; segsum += emask^T @ 1; out += emask^T @ x_chunk.
Group epilogue: rs = 1/max(segsum,tiny); out_rows *= rs at PSUM->SBUF flush;
alphas = e * (mask . broadcast(rs^T)) row-reduce on DVE.
"""

import numpy as np

import concourse.bass as bass
import concourse.mybir as mybir
import concourse.tile as tile
from concourse.bass_utils import run_bass_kernel_spmd
from concourse.masks import make_identity

N_CORES = 8
D = 512
PAD_ID = -1.0e6

f32 = mybir.dt.float32
f32r = mybir.dt.float32r
i32 = mybir.dt.int32
A = mybir.AluOpType
AF = mybir.ActivationFunctionType
AX = mybir.AxisListType


def _split_multi_waits(nc, max_waits=1):
    """This walrus build accepts at most one sem wait per instruction; move
    extra waits onto same-engine NoOps placed immediately before."""
    n = 0
    for fn in nc.m.functions:
        for bb in fn.blocks:
            insts = bb.instructions
            new_list = []
            for inst in insts:
                si = inst.sync_info
                waits = list(si.on_wait) if si and si.on_wait else []
                if len(waits) > max_waits:
                    for w in waits[:-max_waits]:
                        nop = mybir.InstNoOp(
                            name=nc.get_next_instruction_name(), ins=[], outs=[])
                        nop.engine = inst.engine
                        nop.sync_info = mybir.SyncInfo(on_wait=[w], on_update=[])
                        new_list.append(nop)
                        n += 1
                    inst.sync_info = mybir.SyncInfo(
                        on_wait=waits[-max_waits:],
                        on_update=list(si.on_update) if si.on_update else [])
                new_list.append(inst)
            if len(new_list) != len(insts):
                insts[:] = new_list
    return n


def _build_program(G, CH, R):
    """Emit the SPMD per-core program. R = G*CH*128 padded rows per core."""
    nc = bass.Bass("TRN2", target_bir_lowering=False, debug=False)

    xt_d = nc.dram_tensor("xt", (D, R), f32r, kind="ExternalInput").ap()
    x_d = nc.dram_tensor("x", (R, D), f32r, kind="ExternalInput").ap()
    w_d = nc.dram_tensor("w", (D, D), f32r, kind="ExternalInput").ap()
    u_d = nc.dram_tensor("u", (128, 4), f32r, kind="ExternalInput").ap()
    b_d = nc.dram_tensor("b", (128, 4), f32, kind="ExternalInput").ap()
    ids_d = nc.dram_tensor("ids", (128, G * CH), f32, kind="ExternalInput").ap()
    out_d = nc.dram_tensor("out", (G * 128, D), f32, kind="ExternalOutput").ap()
    al_d = nc.dram_tensor("alphas", (128, G * CH), f32, kind="ExternalOutput").ap()

    xt_r = xt_d.rearrange("(ko p) r -> p ko r", p=128)

    with tile.TileContext(nc) as tc:
        with (
            tc.tile_pool(name="const", bufs=1) as cpool,
            tc.tile_pool(name="xtp", bufs=2) as xtp,
            tc.tile_pool(name="xp", bufs=3) as xp,
            tc.tile_pool(name="vtp", bufs=2) as vtp,
            tc.tile_pool(name="mk", bufs=2) as mkp,
            tc.tile_pool(name="work", bufs=3) as pool,
            tc.tile_pool(name="ppre", bufs=1, space="PSUM") as ppre,
            tc.tile_pool(name="pvu", bufs=1, space="PSUM") as pvu,
            tc.tile_pool(name="pec", bufs=1, space="PSUM") as pec,
            tc.tile_pool(name="pseg", bufs=1, space="PSUM") as pseg,
            tc.tile_pool(name="pout", bufs=1, space="PSUM") as pout,
        ):
            # ---- constants ----
            w_sb = cpool.tile([128, 4, D], f32r)
            nc.sync.dma_start(w_sb[:], w_d.rearrange("(ko p) n -> p ko n", p=128))
            u_sb = cpool.tile([128, 4], f32r)
            nc.sync.dma_start(u_sb[:], u_d)
            b_sb = cpool.tile([128, 4], f32)
            nc.sync.dma_start(b_sb[:], b_d)
            ids_sb = cpool.tile([128, G * CH], f32)
            nc.sync.dma_start(ids_sb[:], ids_d)
            iota_i = cpool.tile([128, 128], i32)
            nc.gpsimd.iota(iota_i[:], pattern=[[1, 128]], base=0, channel_multiplier=0)
            iota_f = cpool.tile([128, 128], f32)
            nc.vector.tensor_copy(iota_f[:], iota_i[:])
            ones_f2 = cpool.tile([128, 2], f32)
            nc.vector.memset(ones_f2[:], 1.0)
            ones_r2 = cpool.tile([128, 2], f32r)
            nc.vector.tensor_copy(ones_r2[:], ones_f2[:])
            ones11 = cpool.tile([1, 1], f32)
            nc.vector.memset(ones11[:], 1.0)
            ones128 = cpool.tile([128, 128], f32)
            nc.vector.memset(ones128[:], 1.0)
            ident = cpool.tile([128, 128], f32)
            make_identity(nc, ident[:])

            al_sb = cpool.tile([128, G * CH], f32)

            # block sizes per group (chunks)
            blocks = [4] * (CH // 4)
            if CH % 4:
                blocks.append(CH % 4)

            for g in range(G):
                seg_ps = pseg.tile([128, 2], f32, tag="seg")
                out_ps = pout.tile([128, D], f32, tag="out")
                e_sb = pool.tile([128, CH], f32, tag="e")
                mask_cache = mkp.tile([128, CH, 128], f32r, tag="mask")

                cc = 0  # chunk index within group
                for bs in blocks:
                    rows = bs * 128
                    r0 = (g * CH + cc) * 128  # padded row offset of block
                    xt_blk = xtp.tile([128, 4, 4 * 128], f32r, tag="xt")
                    nc.sync.dma_start(xt_blk[:, :, :rows], xt_r[:, :, r0:r0 + rows])

                    preT = ppre.tile([128, 4, 4 * 128], f32, tag="pre")
                    for dk in range(4):
                        for ko in range(4):
                            nc.tensor.matmul(
                                preT[:, dk, :rows],
                                w_sb[:, ko, dk * 128:(dk + 1) * 128],
                                xt_blk[:, ko, :rows],
                                start=(ko == 0), stop=(ko == 3))

                    vT = vtp.tile([128, 4, 4 * 128], f32r, tag="vT")
                    for dk in range(4):
                        nc.scalar.activation(vT[:, dk, :rows], preT[:, dk, :rows],
                                             AF.Tanh, bias=b_sb[:, dk:dk + 1])

                    vu_ps = pvu.tile([1, 4 * 128], f32, tag="vu")
                    for dk in range(4):
                        nc.tensor.matmul(vu_ps[:, :rows], u_sb[:, dk:dk + 1],
                                         vT[:, dk, :rows],
                                         start=(dk == 0), stop=(dk == 3))
                    e_row = pool.tile([1, 4 * 128], f32, tag="erow")
                    nc.scalar.activation(e_row[:, :rows], vu_ps[:, :rows], AF.Exp)

                    for c in range(bs):
                        col = g * CH + cc + c
                        ecol_ps = pec.tile([128, 1], f32, tag="ecol")
                        nc.tensor.matmul(ecol_ps[:],
                                         e_row[:, c * 128:(c + 1) * 128],
                                         ones11[:], start=True, stop=True)
                        nc.vector.tensor_copy(e_sb[:, cc + c:cc + c + 1], ecol_ps[:])

                        nc.vector.tensor_scalar(
                            out=mask_cache[:, cc + c], in0=iota_f[:],
                            scalar1=ids_sb[:, col:col + 1],
                            scalar2=None, op0=A.is_equal)
                        emask = pool.tile([128, 128], f32r, tag="emask")
                        nc.vector.tensor_scalar_mul(
                            emask[:], mask_cache[:, cc + c],
                            e_sb[:, cc + c:cc + c + 1])

                        first = (cc + c == 0)
                        last = (cc + c == CH - 1)
                        nc.tensor.matmul(seg_ps[:], emask[:], ones_r2[:],
                                         start=first, stop=last)
                        x_t = xp.tile([128, D], f32r, tag="x")
                        nc.sync.dma_start(
                            x_t[:], x_d[(g * CH + cc + c) * 128:
                                        (g * CH + cc + c + 1) * 128, :])
                        nc.tensor.matmul(out_ps[:], emask[:], x_t[:],
                                         start=first, stop=last)
                    cc += bs

                # ---- group epilogue ----
                seg_sb = pool.tile([128, 1], f32, tag="segsb")
                nc.vector.tensor_scalar_max(seg_sb[:], seg_ps[:, 0:1], 1e-30)
                rs_sb = pool.tile([128, 1], f32, tag="rs")
                nc.vector.reciprocal(rs_sb[:], seg_sb[:])

                o_sb = pool.tile([128, D], f32, tag="osb")
                nc.scalar.activation(o_sb[:], out_ps[:], AF.Copy, scale=rs_sb[:])
                nc.sync.dma_start(out_d[g * 128:(g + 1) * 128, :], o_sb[:])

                rs_rep = pool.tile([128, 128], f32, tag="rsrep")
                nc.vector.tensor_scalar_mul(rs_rep[:], ones128[:], rs_sb[:])
                rsT_ps = ppre.tile([128, 128], f32, tag="pre")
                nc.tensor.transpose(rsT_ps[:], rs_rep[:], ident[:])
                rs_bc = pool.tile([128, 128], f32, tag="rsbc")
                nc.vector.tensor_copy(rs_bc[:], rsT_ps[:])

                for c in range(CH):
                    col = g * CH + c
                    g1 = pool.tile([128, 128], f32, tag="g1")
                    nc.vector.tensor_tensor(g1[:], mask_cache[:, c], rs_bc[:], A.mult)
                    gath = pool.tile([128, 1], f32, tag="gath")
                    nc.vector.reduce_sum(gath[:], g1[:], axis=AX.X)
                    nc.vector.tensor_tensor(al_sb[:, col:col + 1],
                                            e_sb[:, c:c + 1], gath[:], A.mult)

            nc.sync.dma_start(al_d, al_sb[:])

    _split_multi_waits(nc)
    return nc


_PROGRAM_CACHE = {}


def _get_program(G, CH, R):
    key = (G, CH, R)
    if key not in _PROGRAM_CACHE:
        _PROGRAM_CACHE[key] = _build_program(G, CH, R)
    return _PROGRAM_CACHE[key]


def kernel(x, w_omega, b_omega, u_omega, type_batch_idx, num_segments,
           _want_trace=False):
    x = np.asarray(x, dtype=np.float32)
    w_omega = np.asarray(w_omega, dtype=np.float32)
    b_omega = np.asarray(b_omega, dtype=np.float32).reshape(-1)
    u_omega = np.asarray(u_omega, dtype=np.float32).reshape(-1)
    ids = np.asarray(type_batch_idx).astype(np.int64).reshape(-1)
    S = int(num_segments)
    N = x.shape[0]
    assert x.shape[1] == D and w_omega.shape == (D, D)

    S_per = -(-S // N_CORES)          # segments per core
    G = -(-S_per // 128)              # 128-segment groups per core

    # row range per core (ids sorted): rows with id in [c*S_per, (c+1)*S_per)
    core_lo = np.searchsorted(ids, np.arange(N_CORES) * S_per, side="left")
    core_hi = np.searchsorted(ids, (np.arange(N_CORES) + 1) * S_per, side="left")

    # group row ranges per core, and the global max group size -> CH
    grp_ranges = np.empty((N_CORES, G + 1), dtype=np.int64)
    for c in range(N_CORES):
        local = ids[core_lo[c]:core_hi[c]] - c * S_per
        grp_ranges[c] = core_lo[c] + np.searchsorted(
            local, np.arange(G + 1) * 128, side="left")
    grp_sizes = np.diff(grp_ranges, axis=1)
    CH = max(1, int(-(-grp_sizes.max() // 128)))
    R = G * CH * 128

    nc = _get_program(G, CH, R)

    u4 = np.ascontiguousarray(u_omega.reshape(4, 128).T)
    b4 = np.ascontiguousarray(b_omega.reshape(4, 128).T)

    def pack_core(c):
        x_pad = np.zeros((R, D), dtype=np.float32)
        ids_rel = np.full(R, PAD_ID, dtype=np.float32)
        row_src = np.full(R, -1, dtype=np.int64)
        for g in range(G):
            lo, hi = grp_ranges[c, g], grp_ranges[c, g + 1]
            n = hi - lo
            off = g * CH * 128
            if n:
                x_pad[off:off + n] = x[lo:hi]
                ids_rel[off:off + n] = (ids[lo:hi] - c * S_per - g * 128).astype(
                    np.float32)
                row_src[off:off + n] = np.arange(lo, hi)
        in_map = {
            "xt": np.ascontiguousarray(x_pad.T),
            "x": x_pad,
            "w": w_omega,
            "u": u4,
            "b": b4,
            "ids": np.ascontiguousarray(ids_rel.reshape(G * CH, 128).T),
        }
        return in_map, row_src

    from concurrent.futures import ThreadPoolExecutor
    with ThreadPoolExecutor(max_workers=N_CORES) as ex:
        packed = list(ex.map(pack_core, range(N_CORES)))
    in_maps = [p[0] for p in packed]
    row_srcs = [p[1] for p in packed]

    kwargs = {}
    if _want_trace:
        kwargs = dict(trace=True, trace_cores=list(range(N_CORES)))
    res = run_bass_kernel_spmd(nc, in_maps, core_ids=list(range(N_CORES)),
                               **kwargs)

    output = np.zeros((S, D), dtype=np.float32)
    alphas = np.zeros(N, dtype=np.float32)
    for c in range(N_CORES):
        s0 = c * S_per
        n_seg = min(S_per, S - s0)
        output[s0:s0 + n_seg] = res.results[c]["out"][:n_seg]
        al_flat = np.ascontiguousarray(res.results[c]["alphas"].T).reshape(-1)
        valid = row_srcs[c] >= 0
        alphas[row_srcs[c][valid]] = al_flat[valid]

    out_tuple = (output, alphas.reshape(N, 1))
    if _want_trace:
        return out_tuple, res
    return out_tuple


# revision 2
# speedup vs baseline: 425.5148x; 425.5148x over previous
"""Segment-softmax attention-scatter kernel for 8 Trainium2 NeuronCores.

Math (per reference):
    v = tanh(x @ W + b);  vu = v @ u;  e = exp(vu)        [no max-subtract:
        |vu| <= ||u||_1 ~ 28, exp never overflows fp32]
    alphas = e / segment_sum(e);  out = segment_sum(x * alphas)

Sharding: segments are split into 8 contiguous ranges (ids are sorted, so each
core's rows are one contiguous slice -> no cross-core reduction at all). Each
core owns G=ceil(S/8/128) groups of 128 segments. Host pads every (core,group)
row range to CH chunks of 128 rows so all 8 cores run one identical NEFF.

Per 512-row block (transposed phase A):
    preT[d',r] = sum_d W[d,d'] x[r,d]   (16 fp32r matmuls, W stationary)
    vT = tanh(preT + b)                  (ACT, bias is per-partition here)
    vu[1,r] += u_dk^T @ vT_dk            (4 fp32r matmuls)
    e_row = exp(vu)                      (ACT)
Per 128-row chunk:
    e_col = e_row-slice^T via K=1 matmul; mask = (iota == id-128g) on DVE;
    emask = mask * e_col; segsum += emask^T @ 1; out += emask^T @ x_chunk.
Group epilogue: rs = 1/max(segsum,tiny); out_rows *= rs at PSUM->SBUF flush;
alphas = e * (mask . broadcast(rs^T)) row-reduce on DVE.
"""

import numpy as np

import concourse.bass as bass
import concourse.mybir as mybir
import concourse.tile as tile
from concourse.bass_utils import run_bass_kernel_spmd
from concourse.masks import make_identity

N_CORES = 8
D = 512
PAD_ID = -1.0e6

f32 = mybir.dt.float32
f32r = mybir.dt.float32r
i32 = mybir.dt.int32
A = mybir.AluOpType
AF = mybir.ActivationFunctionType
AX = mybir.AxisListType


def _split_multi_waits(nc, max_waits=1):
    """This walrus build accepts at most one sem wait per instruction; move
    extra waits onto same-engine NoOps placed immediately before."""
    n = 0
    for fn in nc.m.functions:
        for bb in fn.blocks:
            insts = bb.instructions
            new_list = []
            for inst in insts:
                si = inst.sync_info
                waits = list(si.on_wait) if si and si.on_wait else []
                if len(waits) > max_waits:
                    for w in waits[:-max_waits]:
                        nop = mybir.InstNoOp(
                            name=nc.get_next_instruction_name(), ins=[], outs=[])
                        nop.engine = inst.engine
                        nop.sync_info = mybir.SyncInfo(on_wait=[w], on_update=[])
                        new_list.append(nop)
                        n += 1
                    inst.sync_info = mybir.SyncInfo(
                        on_wait=waits[-max_waits:],
                        on_update=list(si.on_update) if si.on_update else [])
                new_list.append(inst)
            if len(new_list) != len(insts):
                insts[:] = new_list
    return n


def _build_program(G, CH, R):
    """Emit the SPMD per-core program. R = G*CH*128 padded rows per core."""
    nc = bass.Bass("TRN2", target_bir_lowering=False, debug=False)

    xt_d = nc.dram_tensor("xt", (D, R), f32r, kind="ExternalInput").ap()
    x_d = nc.dram_tensor("x", (R, D), f32r, kind="ExternalInput").ap()
    w_d = nc.dram_tensor("w", (D, D), f32r, kind="ExternalInput").ap()
    u_d = nc.dram_tensor("u", (128, 4), f32r, kind="ExternalInput").ap()
    b_d = nc.dram_tensor("b", (128, 4), f32, kind="ExternalInput").ap()
    ids_d = nc.dram_tensor("ids", (128, G * CH), f32, kind="ExternalInput").ap()
    out_d = nc.dram_tensor("out", (G * 128, D), f32, kind="ExternalOutput").ap()
    al_d = nc.dram_tensor("alphas", (128, G * CH), f32, kind="ExternalOutput").ap()

    xt_r = xt_d.rearrange("(ko p) r -> p ko r", p=128)

    with tile.TileContext(nc) as tc:
        with (
            tc.tile_pool(name="const", bufs=1) as cpool,
            tc.tile_pool(name="xtp", bufs=2) as xtp,
            tc.tile_pool(name="xp", bufs=3) as xp,
            tc.tile_pool(name="vtp", bufs=2) as vtp,
            tc.tile_pool(name="mk", bufs=2) as mkp,
            tc.tile_pool(name="work", bufs=3) as pool,
            tc.tile_pool(name="ppre", bufs=1, space="PSUM") as ppre,
            tc.tile_pool(name="pvu", bufs=1, space="PSUM") as pvu,
            tc.tile_pool(name="pec", bufs=1, space="PSUM") as pec,
            tc.tile_pool(name="pseg", bufs=1, space="PSUM") as pseg,
            tc.tile_pool(name="pout", bufs=1, space="PSUM") as pout,
        ):
            # ---- constants ----
            w_sb = cpool.tile([128, 4, D], f32r)
            nc.sync.dma_start(w_sb[:], w_d.rearrange("(ko p) n -> p ko n", p=128))
            u_sb = cpool.tile([128, 4], f32r)
            nc.sync.dma_start(u_sb[:], u_d)
            b_sb = cpool.tile([128, 4], f32)
            nc.sync.dma_start(b_sb[:], b_d)
            ids_sb = cpool.tile([128, G * CH], f32)
            nc.sync.dma_start(ids_sb[:], ids_d)
            iota_i = cpool.tile([128, 128], i32)
            nc.gpsimd.iota(iota_i[:], pattern=[[1, 128]], base=0, channel_multiplier=0)
            iota_f = cpool.tile([128, 128], f32)
            nc.vector.tensor_copy(iota_f[:], iota_i[:])
            ones_f2 = cpool.tile([128, 2], f32)
            nc.vector.memset(ones_f2[:], 1.0)
            ones_r2 = cpool.tile([128, 2], f32r)
            nc.vector.tensor_copy(ones_r2[:], ones_f2[:])
            ones11 = cpool.tile([1, 1], f32)
            nc.vector.memset(ones11[:], 1.0)
            ones128 = cpool.tile([128, 128], f32)
            nc.vector.memset(ones128[:], 1.0)
            ident = cpool.tile([128, 128], f32)
            make_identity(nc, ident[:])

            al_sb = cpool.tile([128, G * CH], f32)

            # block sizes per group (chunks)
            blocks = [4] * (CH // 4)
            if CH % 4:
                blocks.append(CH % 4)

            for g in range(G):
                seg_ps = pseg.tile([128, 2], f32, tag="seg")
                out_ps = pout.tile([128, D], f32, tag="out")
                e_sb = pool.tile([128, CH], f32, tag="e")
                mask_cache = mkp.tile([128, CH, 128], f32r, tag="mask")

                cc = 0  # chunk index within group
                for bs in blocks:
                    rows = bs * 128
                    r0 = (g * CH + cc) * 128  # padded row offset of block
                    xt_blk = xtp.tile([128, 4, 4 * 128], f32r, tag="xt")
                    nc.sync.dma_start(xt_blk[:, :, :rows], xt_r[:, :, r0:r0 + rows])

                    preT = ppre.tile([128, 4, 4 * 128], f32, tag="pre")
                    for dk in range(4):
                        for ko in range(4):
                            nc.tensor.matmul(
                                preT[:, dk, :rows],
                                w_sb[:, ko, dk * 128:(dk + 1) * 128],
                                xt_blk[:, ko, :rows],
                                start=(ko == 0), stop=(ko == 3))

                    vT = vtp.tile([128, 4, 4 * 128], f32r, tag="vT")
                    for dk in range(4):
                        nc.scalar.activation(vT[:, dk, :rows], preT[:, dk, :rows],
                                             AF.Tanh, bias=b_sb[:, dk:dk + 1])

                    vu_ps = pvu.tile([1, 4 * 128], f32, tag="vu")
                    for dk in range(4):
                        nc.tensor.matmul(vu_ps[:, :rows], u_sb[:, dk:dk + 1],
                                         vT[:, dk, :rows],
                                         start=(dk == 0), stop=(dk == 3))
                    e_row = pool.tile([1, 4 * 128], f32, tag="erow")
                    nc.scalar.activation(e_row[:, :rows], vu_ps[:, :rows], AF.Exp)

                    for c in range(bs):
                        col = g * CH + cc + c
                        ecol_ps = pec.tile([128, 1], f32, tag="ecol")
                        nc.tensor.matmul(ecol_ps[:],
                                         e_row[:, c * 128:(c + 1) * 128],
                                         ones11[:], start=True, stop=True)
                        nc.vector.tensor_copy(e_sb[:, cc + c:cc + c + 1], ecol_ps[:])

                        nc.vector.tensor_scalar(
                            out=mask_cache[:, cc + c], in0=iota_f[:],
                            scalar1=ids_sb[:, col:col + 1],
                            scalar2=None, op0=A.is_equal)
                        emask = pool.tile([128, 128], f32r, tag="emask")
                        nc.vector.tensor_scalar_mul(
                            emask[:], mask_cache[:, cc + c],
                            e_sb[:, cc + c:cc + c + 1])

                        first = (cc + c == 0)
                        last = (cc + c == CH - 1)
                        nc.tensor.matmul(seg_ps[:], emask[:], ones_r2[:],
                                         start=first, stop=last)
                        x_t = xp.tile([128, D], f32r, tag="x")
                        nc.sync.dma_start(
                            x_t[:], x_d[(g * CH + cc + c) * 128:
                                        (g * CH + cc + c + 1) * 128, :])
                        nc.tensor.matmul(out_ps[:], emask[:], x_t[:],
                                         start=first, stop=last)
                    cc += bs

                # ---- group epilogue ----
                seg_sb = pool.tile([128, 1], f32, tag="segsb")
                nc.vector.tensor_scalar_max(seg_sb[:], seg_ps[:, 0:1], 1e-30)
                rs_sb = pool.tile([128, 1], f32, tag="rs")
                nc.vector.reciprocal(rs_sb[:], seg_sb[:])

                o_sb = pool.tile([128, D], f32, tag="osb")
                nc.scalar.activation(o_sb[:], out_ps[:], AF.Copy, scale=rs_sb[:])
                nc.sync.dma_start(out_d[g * 128:(g + 1) * 128, :], o_sb[:])

                rs_rep = pool.tile([128, 128], f32, tag="rsrep")
                nc.vector.tensor_scalar_mul(rs_rep[:], ones128[:], rs_sb[:])
                rsT_ps = ppre.tile([128, 128], f32, tag="pre")
                nc.tensor.transpose(rsT_ps[:], rs_rep[:], ident[:])
                rs_bc = pool.tile([128, 128], f32, tag="rsbc")
                nc.vector.tensor_copy(rs_bc[:], rsT_ps[:])

                for c in range(CH):
                    col = g * CH + c
                    g1 = pool.tile([128, 128], f32, tag="g1")
                    nc.vector.tensor_tensor(g1[:], mask_cache[:, c], rs_bc[:], A.mult)
                    gath = pool.tile([128, 1], f32, tag="gath")
                    nc.vector.reduce_sum(gath[:], g1[:], axis=AX.X)
                    nc.vector.tensor_tensor(al_sb[:, col:col + 1],
                                            e_sb[:, c:c + 1], gath[:], A.mult)

            nc.sync.dma_start(al_d, al_sb[:])

    _split_multi_waits(nc)
    return nc


_PROGRAM_CACHE = {}


def _get_program(G, CH, R):
    key = (G, CH, R)
    if key not in _PROGRAM_CACHE:
        _PROGRAM_CACHE[key] = _build_program(G, CH, R)
    return _PROGRAM_CACHE[key]


def prepare(x, w_omega, b_omega, u_omega, type_batch_idx, num_segments):
    x = np.asarray(x, dtype=np.float32)
    w_omega = np.asarray(w_omega, dtype=np.float32)
    b_omega = np.asarray(b_omega, dtype=np.float32).reshape(-1)
    u_omega = np.asarray(u_omega, dtype=np.float32).reshape(-1)
    ids = np.asarray(type_batch_idx).astype(np.int64).reshape(-1)
    S = int(num_segments)
    N = x.shape[0]
    assert x.shape[1] == D and w_omega.shape == (D, D)

    S_per = -(-S // N_CORES)          # segments per core
    G = -(-S_per // 128)              # 128-segment groups per core

    # row range per core (ids sorted): rows with id in [c*S_per, (c+1)*S_per)
    core_lo = np.searchsorted(ids, np.arange(N_CORES) * S_per, side="left")
    core_hi = np.searchsorted(ids, (np.arange(N_CORES) + 1) * S_per, side="left")

    # group row ranges per core, and the global max group size -> CH
    grp_ranges = np.empty((N_CORES, G + 1), dtype=np.int64)
    for c in range(N_CORES):
        local = ids[core_lo[c]:core_hi[c]] - c * S_per
        grp_ranges[c] = core_lo[c] + np.searchsorted(
            local, np.arange(G + 1) * 128, side="left")
    grp_sizes = np.diff(grp_ranges, axis=1)
    CH = max(1, int(-(-grp_sizes.max() // 128)))
    R = G * CH * 128

    nc = _get_program(G, CH, R)

    u4 = np.ascontiguousarray(u_omega.reshape(4, 128).T)
    b4 = np.ascontiguousarray(b_omega.reshape(4, 128).T)

    def pack_core(c):
        x_pad = np.zeros((R, D), dtype=np.float32)
        ids_rel = np.full(R, PAD_ID, dtype=np.float32)
        row_src = np.full(R, -1, dtype=np.int64)
        for g in range(G):
            lo, hi = grp_ranges[c, g], grp_ranges[c, g + 1]
            n = hi - lo
            off = g * CH * 128
            if n:
                x_pad[off:off + n] = x[lo:hi]
                ids_rel[off:off + n] = (ids[lo:hi] - c * S_per - g * 128).astype(
                    np.float32)
                row_src[off:off + n] = np.arange(lo, hi)
        in_map = {
            "xt": np.ascontiguousarray(x_pad.T),
            "x": x_pad,
            "w": w_omega,
            "u": u4,
            "b": b4,
            "ids": np.ascontiguousarray(ids_rel.reshape(G * CH, 128).T),
        }
        return in_map, row_src

    from concurrent.futures import ThreadPoolExecutor
    with ThreadPoolExecutor(max_workers=N_CORES) as ex:
        packed = list(ex.map(pack_core, range(N_CORES)))
    in_maps = [p[0] for p in packed]
    row_srcs = [p[1] for p in packed]
    meta = {"S": S, "N": N, "S_per": S_per, "G": G, "CH": CH, "R": R,
            "row_srcs": row_srcs}
    return nc, in_maps, meta


def unshard(results, meta):
    S, N, S_per = meta["S"], meta["N"], meta["S_per"]
    output = np.zeros((S, D), dtype=np.float32)
    alphas = np.zeros(N, dtype=np.float32)
    for c in range(N_CORES):
        s0 = c * S_per
        n_seg = min(S_per, S - s0)
        output[s0:s0 + n_seg] = results[c]["out"][:n_seg]
        al_flat = np.ascontiguousarray(results[c]["alphas"].T).reshape(-1)
        valid = meta["row_srcs"][c] >= 0
        alphas[meta["row_srcs"][c][valid]] = al_flat[valid]
    return output, alphas.reshape(N, 1)


def kernel(x, w_omega, b_omega, u_omega, type_batch_idx, num_segments):
    nc, in_maps, meta = prepare(x, w_omega, b_omega, u_omega,
                                type_batch_idx, num_segments)
    res = run_bass_kernel_spmd(nc, in_maps, core_ids=list(range(N_CORES)))
    return unshard(res.results, meta)


# revision 3
# speedup vs baseline: 127222.2683x; 298.9844x over previous
"""Segment-softmax attention-scatter kernel for 8 Trainium2 NeuronCores.

Math (per reference):
    v = tanh(x @ W + b);  vu = v @ u;  e = exp(vu)        [no max-subtract:
        |vu| <= ||u||_1 ~ 28, exp never overflows fp32]
    alphas = e / segment_sum(e);  out = segment_sum(x * alphas)

Sharding: segments are split into 8 contiguous ranges (ids are sorted, so each
core's rows are one contiguous slice -> no cross-core reduction at all). Each
core owns G=ceil(S/8/128) groups of 128 segments. Host pads every (core,group)
row range to CH chunks of 128 rows so all 8 cores run one identical NEFF.

Per 512-row block (transposed phase A):
    preT[d',r] = sum_d W[d,d'] x[r,d]   (16 fp32r matmuls, W stationary)
    vT = tanh(preT + b)                  (ACT, bias is per-partition here)
    vu[1,r] += u_dk^T @ vT_dk            (4 fp32r matmuls)
    e_row = exp(vu)                      (ACT)
Per 128-row chunk:
    e_col = e_row-slice^T via K=1 matmul; mask = (iota == id-128g) on DVE;
    emask = mask * e_col; segsum += emask^T @ 1; out += emask^T @ x_chunk.
Group epilogue: rs = 1/max(segsum,tiny); out_rows *= rs at PSUM->SBUF flush;
alphas = e * (mask . broadcast(rs^T)) row-reduce on DVE.
"""

import numpy as np

import concourse.bass as bass
import concourse.mybir as mybir
import concourse.tile as tile
from concourse.bass_utils import run_bass_kernel_spmd
from concourse.masks import make_identity

N_CORES = 8
D = 512
PAD_ID = -1.0e6

f32 = mybir.dt.float32
f32r = mybir.dt.float32r
i32 = mybir.dt.int32
A = mybir.AluOpType
AF = mybir.ActivationFunctionType
AX = mybir.AxisListType


def _split_multi_waits(nc, max_waits=1):
    """This walrus build accepts at most one sem wait per instruction; move
    extra waits onto same-engine NoOps placed immediately before."""
    n = 0
    for fn in nc.m.functions:
        for bb in fn.blocks:
            insts = bb.instructions
            new_list = []
            for inst in insts:
                si = inst.sync_info
                waits = list(si.on_wait) if si and si.on_wait else []
                if len(waits) > max_waits:
                    for w in waits[:-max_waits]:
                        nop = mybir.InstNoOp(
                            name=nc.get_next_instruction_name(), ins=[], outs=[])
                        nop.engine = inst.engine
                        nop.sync_info = mybir.SyncInfo(on_wait=[w], on_update=[])
                        new_list.append(nop)
                        n += 1
                    inst.sync_info = mybir.SyncInfo(
                        on_wait=waits[-max_waits:],
                        on_update=list(si.on_update) if si.on_update else [])
                new_list.append(inst)
            if len(new_list) != len(insts):
                insts[:] = new_list
    return n


def _build_program(G, CH, R, rep=1):
    """Emit the SPMD per-core program. R = G*CH*128 padded rows per core."""
    nc = bass.Bass("TRN2", target_bir_lowering=False, debug=False)

    xt_d = nc.dram_tensor("xt", (D, R), f32r, kind="ExternalInput").ap()
    x_d = nc.dram_tensor("x", (R, D), f32r, kind="ExternalInput").ap()
    w_d = nc.dram_tensor("w", (D, D), f32r, kind="ExternalInput").ap()
    u_d = nc.dram_tensor("u", (128, 4), f32r, kind="ExternalInput").ap()
    b_d = nc.dram_tensor("b", (128, 4), f32, kind="ExternalInput").ap()
    ids_d = nc.dram_tensor("ids", (128, G * CH), f32, kind="ExternalInput").ap()
    out_d = nc.dram_tensor("out", (G * 128, D), f32, kind="ExternalOutput").ap()
    al_d = nc.dram_tensor("alphas", (128, G * CH), f32, kind="ExternalOutput").ap()

    xt_r = xt_d.rearrange("(ko p) r -> p ko r", p=128)

    with tile.TileContext(nc) as tc:
        with (
            tc.tile_pool(name="const", bufs=1) as cpool,
            tc.tile_pool(name="xtp", bufs=2) as xtp,
            tc.tile_pool(name="xp", bufs=3) as xp,
            tc.tile_pool(name="vtp", bufs=2) as vtp,
            tc.tile_pool(name="mk", bufs=2) as mkp,
            tc.tile_pool(name="work", bufs=3) as pool,
            tc.tile_pool(name="ppre", bufs=1, space="PSUM") as ppre,
            tc.tile_pool(name="pvu", bufs=1, space="PSUM") as pvu,
            tc.tile_pool(name="pec", bufs=1, space="PSUM") as pec,
            tc.tile_pool(name="pseg", bufs=1, space="PSUM") as pseg,
            tc.tile_pool(name="pout", bufs=1, space="PSUM") as pout,
        ):
            # ---- constants ----
            w_sb = cpool.tile([128, 4, D], f32r)
            nc.sync.dma_start(w_sb[:], w_d.rearrange("(ko p) n -> p ko n", p=128))
            u_sb = cpool.tile([128, 4], f32r)
            nc.sync.dma_start(u_sb[:], u_d)
            b_sb = cpool.tile([128, 4], f32)
            nc.sync.dma_start(b_sb[:], b_d)
            ids_sb = cpool.tile([128, G * CH], f32)
            nc.sync.dma_start(ids_sb[:], ids_d)
            iota_i = cpool.tile([128, 128], i32)
            nc.gpsimd.iota(iota_i[:], pattern=[[1, 128]], base=0, channel_multiplier=0)
            iota_f = cpool.tile([128, 128], f32)
            nc.vector.tensor_copy(iota_f[:], iota_i[:])
            ones_f2 = cpool.tile([128, 2], f32)
            nc.vector.memset(ones_f2[:], 1.0)
            ones_r2 = cpool.tile([128, 2], f32r)
            nc.vector.tensor_copy(ones_r2[:], ones_f2[:])
            ones11 = cpool.tile([1, 1], f32)
            nc.vector.memset(ones11[:], 1.0)
            ones128 = cpool.tile([128, 128], f32)
            nc.vector.memset(ones128[:], 1.0)
            ident = cpool.tile([128, 128], f32)
            make_identity(nc, ident[:])

            al_sb = cpool.tile([128, G * CH], f32)

            # block sizes per group (chunks)
            blocks = [4] * (CH // 4)
            if CH % 4:
                blocks.append(CH % 4)

            for g in [gg for _ in range(rep) for gg in range(G)]:
                seg_ps = pseg.tile([128, 2], f32, tag="seg")
                out_ps = pout.tile([128, D], f32, tag="out")
                e_sb = pool.tile([128, CH], f32, tag="e")
                mask_cache = mkp.tile([128, CH, 128], f32r, tag="mask")

                cc = 0  # chunk index within group
                for bs in blocks:
                    rows = bs * 128
                    r0 = (g * CH + cc) * 128  # padded row offset of block
                    xt_blk = xtp.tile([128, 4, 4 * 128], f32r, tag="xt")
                    nc.sync.dma_start(xt_blk[:, :, :rows], xt_r[:, :, r0:r0 + rows])

                    preT = ppre.tile([128, 4, 4 * 128], f32, tag="pre")
                    for dk in range(4):
                        for ko in range(4):
                            nc.tensor.matmul(
                                preT[:, dk, :rows],
                                w_sb[:, ko, dk * 128:(dk + 1) * 128],
                                xt_blk[:, ko, :rows],
                                start=(ko == 0), stop=(ko == 3))

                    vT = vtp.tile([128, 4, 4 * 128], f32r, tag="vT")
                    for dk in range(4):
                        nc.scalar.activation(vT[:, dk, :rows], preT[:, dk, :rows],
                                             AF.Tanh, bias=b_sb[:, dk:dk + 1])

                    vu_ps = pvu.tile([1, 4 * 128], f32, tag="vu")
                    for dk in range(4):
                        nc.tensor.matmul(vu_ps[:, :rows], u_sb[:, dk:dk + 1],
                                         vT[:, dk, :rows],
                                         start=(dk == 0), stop=(dk == 3))
                    e_row = pool.tile([1, 4 * 128], f32, tag="erow")
                    nc.scalar.activation(e_row[:, :rows], vu_ps[:, :rows], AF.Exp)

                    for c in range(bs):
                        col = g * CH + cc + c
                        ecol_ps = pec.tile([128, 1], f32, tag="ecol")
                        nc.tensor.matmul(ecol_ps[:],
                                         e_row[:, c * 128:(c + 1) * 128],
                                         ones11[:], start=True, stop=True)
                        nc.vector.tensor_copy(e_sb[:, cc + c:cc + c + 1], ecol_ps[:])

                        nc.vector.tensor_scalar(
                            out=mask_cache[:, cc + c], in0=iota_f[:],
                            scalar1=ids_sb[:, col:col + 1],
                            scalar2=None, op0=A.is_equal)
                        emask = pool.tile([128, 128], f32r, tag="emask")
                        nc.vector.tensor_scalar_mul(
                            emask[:], mask_cache[:, cc + c],
                            e_sb[:, cc + c:cc + c + 1])

                        first = (cc + c == 0)
                        last = (cc + c == CH - 1)
                        nc.tensor.matmul(seg_ps[:], emask[:], ones_r2[:],
                                         start=first, stop=last)
                        x_t = xp.tile([128, D], f32r, tag="x")
                        nc.sync.dma_start(
                            x_t[:], x_d[(g * CH + cc + c) * 128:
                                        (g * CH + cc + c + 1) * 128, :])
                        nc.tensor.matmul(out_ps[:], emask[:], x_t[:],
                                         start=first, stop=last)
                    cc += bs

                # ---- group epilogue ----
                seg_sb = pool.tile([128, 1], f32, tag="segsb")
                nc.vector.tensor_scalar_max(seg_sb[:], seg_ps[:, 0:1], 1e-30)
                rs_sb = pool.tile([128, 1], f32, tag="rs")
                nc.vector.reciprocal(rs_sb[:], seg_sb[:])

                o_sb = pool.tile([128, D], f32, tag="osb")
                nc.scalar.activation(o_sb[:], out_ps[:], AF.Copy, scale=rs_sb[:])
                nc.sync.dma_start(out_d[g * 128:(g + 1) * 128, :], o_sb[:])

                rs_rep = pool.tile([128, 128], f32, tag="rsrep")
                nc.vector.tensor_scalar_mul(rs_rep[:], ones128[:], rs_sb[:])
                rsT_ps = ppre.tile([128, 128], f32, tag="pre")
                nc.tensor.transpose(rsT_ps[:], rs_rep[:], ident[:])
                rs_bc = pool.tile([128, 128], f32, tag="rsbc")
                nc.vector.tensor_copy(rs_bc[:], rsT_ps[:])

                for c in range(CH):
                    col = g * CH + c
                    g1 = pool.tile([128, 128], f32, tag="g1")
                    nc.vector.tensor_tensor(g1[:], mask_cache[:, c], rs_bc[:], A.mult)
                    gath = pool.tile([128, 1], f32, tag="gath")
                    nc.vector.reduce_sum(gath[:], g1[:], axis=AX.X)
                    nc.vector.tensor_tensor(al_sb[:, col:col + 1],
                                            e_sb[:, c:c + 1], gath[:], A.mult)

            nc.sync.dma_start(al_d, al_sb[:])

    _split_multi_waits(nc)
    return nc


_PROGRAM_CACHE = {}


def _get_program(G, CH, R, rep=1):
    key = (G, CH, R, rep)
    if key not in _PROGRAM_CACHE:
        _PROGRAM_CACHE[key] = _build_program(G, CH, R, rep)
    return _PROGRAM_CACHE[key]


def prepare(x, w_omega, b_omega, u_omega, type_batch_idx, num_segments,
            rep=1):
    x = np.asarray(x, dtype=np.float32)
    w_omega = np.asarray(w_omega, dtype=np.float32)
    b_omega = np.asarray(b_omega, dtype=np.float32).reshape(-1)
    u_omega = np.asarray(u_omega, dtype=np.float32).reshape(-1)
    ids = np.asarray(type_batch_idx).astype(np.int64).reshape(-1)
    S = int(num_segments)
    N = x.shape[0]
    assert x.shape[1] == D and w_omega.shape == (D, D)

    S_per = -(-S // N_CORES)          # segments per core
    G = -(-S_per // 128)              # 128-segment groups per core

    # row range per core (ids sorted): rows with id in [c*S_per, (c+1)*S_per)
    core_lo = np.searchsorted(ids, np.arange(N_CORES) * S_per, side="left")
    core_hi = np.searchsorted(ids, (np.arange(N_CORES) + 1) * S_per, side="left")

    # group row ranges per core, and the global max group size -> CH
    grp_ranges = np.empty((N_CORES, G + 1), dtype=np.int64)
    for c in range(N_CORES):
        local = ids[core_lo[c]:core_hi[c]] - c * S_per
        grp_ranges[c] = core_lo[c] + np.searchsorted(
            local, np.arange(G + 1) * 128, side="left")
    grp_sizes = np.diff(grp_ranges, axis=1)
    CH = max(1, int(-(-grp_sizes.max() // 128)))
    R = G * CH * 128

    nc = _get_program(G, CH, R, rep)

    u4 = np.ascontiguousarray(u_omega.reshape(4, 128).T)
    b4 = np.ascontiguousarray(b_omega.reshape(4, 128).T)

    def pack_core(c):
        x_pad = np.zeros((R, D), dtype=np.float32)
        ids_rel = np.full(R, PAD_ID, dtype=np.float32)
        row_src = np.full(R, -1, dtype=np.int64)
        for g in range(G):
            lo, hi = grp_ranges[c, g], grp_ranges[c, g + 1]
            n = hi - lo
            off = g * CH * 128
            if n:
                x_pad[off:off + n] = x[lo:hi]
                ids_rel[off:off + n] = (ids[lo:hi] - c * S_per - g * 128).astype(
                    np.float32)
                row_src[off:off + n] = np.arange(lo, hi)
        in_map = {
            "xt": np.ascontiguousarray(x_pad.T),
            "x": x_pad,
            "w": w_omega,
            "u": u4,
            "b": b4,
            "ids": np.ascontiguousarray(ids_rel.reshape(G * CH, 128).T),
        }
        return in_map, row_src

    from concurrent.futures import ThreadPoolExecutor
    with ThreadPoolExecutor(max_workers=N_CORES) as ex:
        packed = list(ex.map(pack_core, range(N_CORES)))
    in_maps = [p[0] for p in packed]
    row_srcs = [p[1] for p in packed]
    meta = {"S": S, "N": N, "S_per": S_per, "G": G, "CH": CH, "R": R,
            "row_srcs": row_srcs}
    return nc, in_maps, meta


def unshard(results, meta):
    S, N, S_per = meta["S"], meta["N"], meta["S_per"]
    output = np.zeros((S, D), dtype=np.float32)
    alphas = np.zeros(N, dtype=np.float32)
    for c in range(N_CORES):
        s0 = c * S_per
        n_seg = min(S_per, S - s0)
        output[s0:s0 + n_seg] = results[c]["out"][:n_seg]
        al_flat = np.ascontiguousarray(results[c]["alphas"].T).reshape(-1)
        valid = meta["row_srcs"][c] >= 0
        alphas[meta["row_srcs"][c][valid]] = al_flat[valid]
    return output, alphas.reshape(N, 1)


def kernel(x, w_omega, b_omega, u_omega, type_batch_idx, num_segments):
    nc, in_maps, meta = prepare(x, w_omega, b_omega, u_omega,
                                type_batch_idx, num_segments)
    res = run_bass_kernel_spmd(nc, in_maps, core_ids=list(range(N_CORES)))
    return unshard(res.results, meta)


# revision 8
# speedup vs baseline: 215187.1761x; 1.6914x over previous
"""Segment-softmax attention-scatter kernel for 8 Trainium2 NeuronCores.

Math (per reference):
    v = tanh(x @ W + b);  vu = v @ u;  e = exp(vu)        [no max-subtract:
        |vu| <= ||u||_1 ~ 28, exp never overflows fp32]
    alphas = e / segment_sum(e);  out = segment_sum(x * alphas)

Sharding: segments are split into 8 contiguous ranges (ids are sorted, so each
core's rows are one contiguous slice -> no cross-core reduction at all). Each
core owns G=ceil(S/8/128) groups of 128 segments. Host pads every (core,group)
row range to CH chunks of 128 rows so all 8 cores run one identical NEFF.

Per 512-row block (transposed phase A):
    preT[d',r] = sum_d W[d,d'] x[r,d]   (16 fp32r matmuls, W stationary)
    vT = tanh(preT + b)                  (ACT, bias is per-partition here)
    vu[1,r] += u_dk^T @ vT_dk            (4 fp32r matmuls)
    e_row = exp(vu)                      (ACT)
Per 128-row chunk:
    e_col = e_row-slice^T via K=1 matmul; mask = (iota == id-128g) on DVE;
    emask = mask * e_col; segsum += emask^T @ 1; out += emask^T @ x_chunk.
Group epilogue: rs = 1/max(segsum,tiny); out_rows *= rs at PSUM->SBUF flush;
alphas = e * (mask . broadcast(rs^T)) row-reduce on DVE.
"""

import numpy as np

import concourse.bass as bass
import concourse.mybir as mybir
import concourse.tile as tile
from concourse.bass_utils import run_bass_kernel_spmd
from concourse.masks import make_identity

N_CORES = 8
D = 512
PAD_ID = -1.0e6
X_BF16 = False  # bf16 x-natural + emask for the scatter matmuls (DMA -25%)
FUSE_TANH = False  # fusing the 4 tanh ops serializes the block pipeline: slower

f32 = mybir.dt.float32
f32r = mybir.dt.float32r
bf16 = mybir.dt.bfloat16
i32 = mybir.dt.int32
A = mybir.AluOpType
AF = mybir.ActivationFunctionType
AX = mybir.AxisListType


def _split_multi_waits(nc, max_waits=1):
    """This walrus build accepts at most one sem wait per instruction; move
    extra waits onto same-engine NoOps placed immediately before."""
    n = 0
    for fn in nc.m.functions:
        for bb in fn.blocks:
            insts = bb.instructions
            new_list = []
            for inst in insts:
                si = inst.sync_info
                waits = list(si.on_wait) if si and si.on_wait else []
                if len(waits) > max_waits:
                    for w in waits[:-max_waits]:
                        nop = mybir.InstNoOp(
                            name=nc.get_next_instruction_name(), ins=[], outs=[])
                        nop.engine = inst.engine
                        nop.sync_info = mybir.SyncInfo(on_wait=[w], on_update=[])
                        new_list.append(nop)
                        n += 1
                    inst.sync_info = mybir.SyncInfo(
                        on_wait=waits[-max_waits:],
                        on_update=list(si.on_update) if si.on_update else [])
                new_list.append(inst)
            if len(new_list) != len(insts):
                insts[:] = new_list
    return n


def _build_program(G, CH, R, rep=1, zero_bias=False):
    """Emit the SPMD per-core program. R = G*CH*128 padded rows per core."""
    nc = bass.Bass("TRN2", target_bir_lowering=False, debug=False)

    xt_d = nc.dram_tensor("xt", (D, R), f32r, kind="ExternalInput").ap()
    xdt = bf16 if X_BF16 else f32r
    x_d = nc.dram_tensor("x", (R, D), xdt, kind="ExternalInput").ap()
    w_d = nc.dram_tensor("w", (D, D), f32r, kind="ExternalInput").ap()
    u_d = nc.dram_tensor("u", (128, 4), f32r, kind="ExternalInput").ap()
    b_d = nc.dram_tensor("b", (128, 4), f32, kind="ExternalInput").ap()
    ids_d = nc.dram_tensor("ids", (128, G * CH), f32, kind="ExternalInput").ap()
    out_d = nc.dram_tensor("out", (G * 128, D), f32, kind="ExternalOutput").ap()
    al_d = nc.dram_tensor("alphas", (128, G * CH), f32, kind="ExternalOutput").ap()

    xt_r = xt_d.rearrange("(ko p) r -> p ko r", p=128)

    with tile.TileContext(nc) as tc:
        with (
            tc.tile_pool(name="const", bufs=1) as cpool,
            tc.tile_pool(name="xtp", bufs=2) as xtp,
            tc.tile_pool(name="xp", bufs=3) as xp,
            tc.tile_pool(name="vtp", bufs=2) as vtp,
            tc.tile_pool(name="mk", bufs=2) as mkp,
            tc.tile_pool(name="work", bufs=3) as pool,
            tc.tile_pool(name="ppre", bufs=1, space="PSUM") as ppre,
            tc.tile_pool(name="pvu", bufs=1, space="PSUM") as pvu,
            tc.tile_pool(name="pec", bufs=1, space="PSUM") as pec,
            tc.tile_pool(name="pseg", bufs=1, space="PSUM") as pseg,
            tc.tile_pool(name="pout", bufs=1, space="PSUM") as pout,
        ):
            # ---- constants ----
            w_sb = cpool.tile([128, 4, D], f32r)
            nc.sync.dma_start(w_sb[:], w_d.rearrange("(ko p) n -> p ko n", p=128))
            u_sb = cpool.tile([128, 4], f32r)
            nc.sync.dma_start(u_sb[:], u_d)
            b_sb = cpool.tile([128, 4], f32)
            nc.sync.dma_start(b_sb[:], b_d)
            ids_sb = cpool.tile([128, G * CH], f32)
            nc.sync.dma_start(ids_sb[:], ids_d)
            iota_i = cpool.tile([128, 128], i32)
            nc.gpsimd.iota(iota_i[:], pattern=[[1, 128]], base=0, channel_multiplier=0)
            iota_f = cpool.tile([128, 128], f32)
            nc.vector.tensor_copy(iota_f[:], iota_i[:])
            ones_f2 = cpool.tile([128, 2], f32)
            nc.vector.memset(ones_f2[:], 1.0)
            ones_r2 = cpool.tile([128, 2], xdt)
            nc.vector.tensor_copy(ones_r2[:], ones_f2[:])
            ones11 = cpool.tile([1, 1], f32)
            nc.vector.memset(ones11[:], 1.0)
            ones128 = cpool.tile([128, 128], f32)
            nc.vector.memset(ones128[:], 1.0)
            ident = cpool.tile([128, 128], f32)
            make_identity(nc, ident[:])

            al_sb = cpool.tile([128, G * CH], f32)

            # block sizes per group (chunks)
            blocks = [4] * (CH // 4)
            if CH % 4:
                blocks.append(CH % 4)

            for g in [gg for _ in range(rep) for gg in range(G)]:
                seg_ps = pseg.tile([128, 2], f32, tag="seg")
                out_ps = pout.tile([128, D], f32, tag="out")
                e_sb = pool.tile([128, CH], f32, tag="e")
                mask_cache = mkp.tile([128, CH, 128], f32r, tag="mask")

                cc = 0  # chunk index within group
                for bs in blocks:
                    rows = bs * 128
                    r0 = (g * CH + cc) * 128  # padded row offset of block
                    xt_blk = xtp.tile([128, 4, 4 * 128], f32r, tag="xt")
                    nc.sync.dma_start(xt_blk[:, :, :rows], xt_r[:, :, r0:r0 + rows])

                    preT = ppre.tile([128, 4, 4 * 128], f32, tag="pre")
                    for dk in range(4):
                        for ko in range(4):
                            nc.tensor.matmul(
                                preT[:, dk, :rows],
                                w_sb[:, ko, dk * 128:(dk + 1) * 128],
                                xt_blk[:, ko, :rows],
                                start=(ko == 0), stop=(ko == 3))

                    vT = vtp.tile([128, 4, 4 * 128], f32r, tag="vT")
                    if zero_bias and FUSE_TANH:
                        nc.scalar.activation(vT[:, :, :rows], preT[:, :, :rows],
                                             AF.Tanh)
                    else:
                        for dk in range(4):
                            nc.scalar.activation(
                                vT[:, dk, :rows], preT[:, dk, :rows],
                                AF.Tanh, bias=b_sb[:, dk:dk + 1])

                    vu_ps = pvu.tile([1, 4 * 128], f32, tag="vu")
                    for dk in range(4):
                        nc.tensor.matmul(vu_ps[:, :rows], u_sb[:, dk:dk + 1],
                                         vT[:, dk, :rows],
                                         start=(dk == 0), stop=(dk == 3))
                    e_row = pool.tile([1, 4 * 128], f32, tag="erow")
                    nc.scalar.activation(e_row[:, :rows], vu_ps[:, :rows], AF.Exp)

                    for c in range(bs):
                        col = g * CH + cc + c
                        ecol_ps = pec.tile([128, 1], f32, tag="ecol")
                        nc.tensor.matmul(ecol_ps[:],
                                         e_row[:, c * 128:(c + 1) * 128],
                                         ones11[:], start=True, stop=True)
                        nc.vector.tensor_copy(e_sb[:, cc + c:cc + c + 1], ecol_ps[:])

                        nc.vector.tensor_scalar(
                            out=mask_cache[:, cc + c], in0=iota_f[:],
                            scalar1=ids_sb[:, col:col + 1],
                            scalar2=None, op0=A.is_equal)
                        emask = pool.tile([128, 128], xdt, tag="emask")
                        nc.vector.tensor_scalar_mul(
                            emask[:], mask_cache[:, cc + c],
                            e_sb[:, cc + c:cc + c + 1])

                        first = (cc + c == 0)
                        last = (cc + c == CH - 1)
                        nc.tensor.matmul(seg_ps[:], emask[:], ones_r2[:],
                                         start=first, stop=last)
                        x_t = xp.tile([128, D], xdt, tag="x")
                        nc.sync.dma_start(
                            x_t[:], x_d[(g * CH + cc + c) * 128:
                                        (g * CH + cc + c + 1) * 128, :])
                        nc.tensor.matmul(out_ps[:], emask[:], x_t[:],
                                         start=first, stop=last)
                    cc += bs

                # ---- group epilogue ----
                seg_sb = pool.tile([128, 1], f32, tag="segsb")
                nc.vector.tensor_scalar_max(seg_sb[:], seg_ps[:, 0:1], 1e-30)
                rs_sb = pool.tile([128, 1], f32, tag="rs")
                nc.vector.reciprocal(rs_sb[:], seg_sb[:])

                o_sb = pool.tile([128, D], f32, tag="osb")
                nc.scalar.activation(o_sb[:], out_ps[:], AF.Copy, scale=rs_sb[:])
                nc.sync.dma_start(out_d[g * 128:(g + 1) * 128, :], o_sb[:])

                rs_rep = pool.tile([128, 128], f32, tag="rsrep")
                nc.vector.tensor_scalar_mul(rs_rep[:], ones128[:], rs_sb[:])
                rsT_ps = ppre.tile([128, 128], f32, tag="pre")
                nc.tensor.transpose(rsT_ps[:], rs_rep[:], ident[:])
                rs_bc = pool.tile([128, 128], f32, tag="rsbc")
                nc.vector.tensor_copy(rs_bc[:], rsT_ps[:])

                for c in range(CH):
                    col = g * CH + c
                    g1 = pool.tile([128, 128], f32, tag="g1")
                    nc.vector.tensor_tensor(g1[:], mask_cache[:, c], rs_bc[:], A.mult)
                    gath = pool.tile([128, 1], f32, tag="gath")
                    nc.vector.reduce_sum(gath[:], g1[:], axis=AX.X)
                    nc.vector.tensor_tensor(al_sb[:, col:col + 1],
                                            e_sb[:, c:c + 1], gath[:], A.mult)

            nc.sync.dma_start(al_d, al_sb[:])

    _split_multi_waits(nc)
    return nc


_PROGRAM_CACHE = {}


def _get_program(G, CH, R, rep=1, zero_bias=False):
    key = (G, CH, R, rep, zero_bias, X_BF16, FUSE_TANH)
    if key not in _PROGRAM_CACHE:
        _PROGRAM_CACHE[key] = _build_program(G, CH, R, rep, zero_bias)
    return _PROGRAM_CACHE[key]


def prepare(x, w_omega, b_omega, u_omega, type_batch_idx, num_segments,
            rep=1):
    x = np.asarray(x, dtype=np.float32)
    w_omega = np.asarray(w_omega, dtype=np.float32)
    b_omega = np.asarray(b_omega, dtype=np.float32).reshape(-1)
    u_omega = np.asarray(u_omega, dtype=np.float32).reshape(-1)
    ids = np.asarray(type_batch_idx).astype(np.int64).reshape(-1)
    S = int(num_segments)
    N = x.shape[0]
    assert x.shape[1] == D and w_omega.shape == (D, D)

    # the reference always provides sorted ids; permute defensively if not
    row_perm = None
    if np.any(np.diff(ids) < 0):
        row_perm = np.argsort(ids, kind="stable")
        ids = ids[row_perm]
        x = x[row_perm]

    S_per = -(-S // N_CORES)          # segments per core
    G = -(-S_per // 128)              # 128-segment groups per core

    # row range per core (ids sorted): rows with id in [c*S_per, (c+1)*S_per)
    core_lo = np.searchsorted(ids, np.arange(N_CORES) * S_per, side="left")
    core_hi = np.searchsorted(ids, (np.arange(N_CORES) + 1) * S_per, side="left")

    # group row ranges per core, and the global max group size -> CH
    grp_ranges = np.empty((N_CORES, G + 1), dtype=np.int64)
    for c in range(N_CORES):
        local = ids[core_lo[c]:core_hi[c]] - c * S_per
        grp_ranges[c] = core_lo[c] + np.searchsorted(
            local, np.arange(G + 1) * 128, side="left")
    grp_sizes = np.diff(grp_ranges, axis=1)
    CH = max(1, int(-(-grp_sizes.max() // 128)))
    R = G * CH * 128

    zero_bias = not np.any(b_omega)
    nc = _get_program(G, CH, R, rep, zero_bias)

    u4 = np.ascontiguousarray(u_omega.reshape(4, 128).T)
    b4 = np.ascontiguousarray(b_omega.reshape(4, 128).T)

    def pack_core(c):
        x_pad = np.zeros((R, D), dtype=np.float32)
        ids_rel = np.full(R, PAD_ID, dtype=np.float32)
        row_src = np.full(R, -1, dtype=np.int64)
        for g in range(G):
            lo, hi = grp_ranges[c, g], grp_ranges[c, g + 1]
            n = hi - lo
            off = g * CH * 128
            if n:
                x_pad[off:off + n] = x[lo:hi]
                ids_rel[off:off + n] = (ids[lo:hi] - c * S_per - g * 128).astype(
                    np.float32)
                row_src[off:off + n] = np.arange(lo, hi)
        if X_BF16:
            import ml_dtypes
            x_n = x_pad.astype(ml_dtypes.bfloat16)
        else:
            x_n = x_pad
        in_map = {
            "xt": np.ascontiguousarray(x_pad.T),
            "x": x_n,
            "w": w_omega,
            "u": u4,
            "b": b4,
            "ids": np.ascontiguousarray(ids_rel.reshape(G * CH, 128).T),
        }
        return in_map, row_src

    from concurrent.futures import ThreadPoolExecutor
    with ThreadPoolExecutor(max_workers=N_CORES) as ex:
        packed = list(ex.map(pack_core, range(N_CORES)))
    in_maps = [p[0] for p in packed]
    row_srcs = [p[1] for p in packed]
    meta = {"S": S, "N": N, "S_per": S_per, "G": G, "CH": CH, "R": R,
            "row_srcs": row_srcs, "row_perm": row_perm}
    return nc, in_maps, meta


def unshard(results, meta):
    S, N, S_per = meta["S"], meta["N"], meta["S_per"]
    output = np.zeros((S, D), dtype=np.float32)
    alphas = np.zeros(N, dtype=np.float32)
    for c in range(N_CORES):
        s0 = c * S_per
        n_seg = min(S_per, S - s0)
        output[s0:s0 + n_seg] = results[c]["out"][:n_seg]
        al_flat = np.ascontiguousarray(results[c]["alphas"].T).reshape(-1)
        valid = meta["row_srcs"][c] >= 0
        alphas[meta["row_srcs"][c][valid]] = al_flat[valid]
    if meta.get("row_perm") is not None:
        un = np.empty_like(alphas)
        un[meta["row_perm"]] = alphas
        alphas = un
    return output, alphas.reshape(N, 1)


def kernel(x, w_omega, b_omega, u_omega, type_batch_idx, num_segments):
    nc, in_maps, meta = prepare(x, w_omega, b_omega, u_omega,
                                type_batch_idx, num_segments)
    res = run_bass_kernel_spmd(nc, in_maps, core_ids=list(range(N_CORES)))
    return unshard(res.results, meta)
